# revision 1
# baseline (speedup 1.0000x reference)
"""GroupedQueryAttention (B=1, S=2048, D=4096, 32 Q heads / 8 KV heads) on 8 TRN2 cores.

Sharding: one KV group (4 Q heads + 1 KV head) per core.  Per core:
  - QKV projection for its head group (q^T/k^T/v^T orientation: dims on partitions)
  - RoPE on q (scale folded in) and k via DVE
  - causal flash-style attention in scores^T orientation:
      scoresT[t,s] tiles from PE, mask-add (diag), exp on ACT -> P^T (bf16),
      denominator = DVE partial-sum accum + ones-matmul partition reduce,
      AV accumulation out^T[d,s] on PE, normalize via reciprocal+ones-broadcast.
  - out-projection partials against OWN 512 Wo rows for ALL 2048 seq rows
    (interleaved per attention chunk), staged to DRAM in bf16;
    one ReduceScatter(add) routes+reduces so core g ends with final
    out[256g:256g+256, :]; host concatenates + bias.

All matmuls bf16 inputs / fp32 PSUM accumulation (measured rel_l2 ~7e-3 vs f64 ref).
"""
import numpy as np
import ml_dtypes

from concourse import bass, bacc, tile, mybir
from concourse.bass_utils import run_bass_kernel_spmd

BF16 = ml_dtypes.bfloat16
F32 = np.float32

D = 4096          # model dim
S = 2048          # sequence
NH = 32           # query heads
NG = 8            # kv heads == n cores
HD = 128          # head dim
G = NH // NG      # 4 query heads per group/core
KV = NG * HD      # 1024
BASE = 50000.0
SCALE = 1.0 / np.sqrt(HD)
N_CORES = 8
SC = S // 512     # 4 s-chunks of 512
MC = D // 128     # 32 contraction chunks
SSLICE = S // N_CORES  # 256 rows of final output per core

_CACHE = {}


def _build(reps: int = 1, sim: bool = False, coll: str = "rs",
           nodma: bool = False, deep: bool = True):
    if sim:
        coll = "copy"

    def dma(*args, **kwargs):
        if not nodma:
            nc.sync.dma_start(*args, **kwargs)
    f32 = mybir.dt.float32
    f32r = mybir.dt.float32r
    bf16 = mybir.dt.bfloat16

    nc = bacc.Bacc("TRN2", target_bir_lowering=False, debug=False,
                   num_devices=N_CORES)

    # ---- I/O ----
    xt_d = nc.dram_tensor("xt", [128, SC * MC * 512], bf16, kind="ExternalInput")
    wq_d = nc.dram_tensor("wq", [128, MC * 512], bf16, kind="ExternalInput")
    wk_d = nc.dram_tensor("wk", [128, MC * 128], bf16, kind="ExternalInput")
    wv_d = nc.dram_tensor("wv", [128, MC * 128], bf16, kind="ExternalInput")
    wo_d = nc.dram_tensor("wo", [128, G * 8 * 512], bf16, kind="ExternalInput")
    cosq_d = nc.dram_tensor("cosq", [128, S], bf16, kind="ExternalInput")
    sinq_d = nc.dram_tensor("sinq", [128, S], bf16, kind="ExternalInput")
    cosk_d = nc.dram_tensor("cosk", [128, S], bf16, kind="ExternalInput")
    sink_d = nc.dram_tensor("sink", [128, S], bf16, kind="ExternalInput")
    # qkv bias as a rank-1 matmul: lhsT [1, 768] (per-block 128-dim slices),
    # rhs = ones row [1, 512]
    bqw_d = nc.dram_tensor("bqw", [1, (G + 2) * 128], bf16, kind="ExternalInput")
    ones_d = nc.dram_tensor("ones", [1, 512], bf16, kind="ExternalInput")
    mask_d = nc.dram_tensor("mask", [128, 4 * 512], bf16, kind="ExternalInput")
    ident_d = nc.dram_tensor("ident", [128, 128], bf16, kind="ExternalInput")
    onem_d = nc.dram_tensor("onem", [128, 128], bf16, kind="ExternalInput")
    out_d = nc.dram_tensor("out", [SSLICE, D], bf16, kind="ExternalOutput")

    Ident = mybir.ActivationFunctionType.Identity
    CopyF = mybir.ActivationFunctionType.Copy
    Exp = mybir.ActivationFunctionType.Exp
    mult = mybir.AluOpType.mult

    with tile.TileContext(nc) as tc:
        with tc.tile_pool(name="const", bufs=1) as cp, \
             tc.tile_pool(name="pers", bufs=1) as pp, \
             tc.tile_pool(name="dram", bufs=1, space="DRAM") as dramp:
            # constants
            bqw = cp.tile([1, (G + 2) * 128], bf16)
            nc.sync.dma_start(bqw[:], bqw_d[:])
            onesr = cp.tile([1, 512], bf16); nc.sync.dma_start(onesr[:], ones_d[:])
            mask = cp.tile([128, 4 * 512], bf16); nc.sync.dma_start(mask[:], mask_d[:])
            ident = cp.tile([128, 128], bf16); nc.sync.dma_start(ident[:], ident_d[:])
            onem = cp.tile([128, 128], bf16); nc.sync.dma_start(onem[:], onem_d[:])

            # persistent per-rep intermediates
            qT = pp.tile([128, G * S], bf16)      # rope'd q^T, head h at [:, h*S:]
            kT = pp.tile([128, S], bf16)
            vN = pp.tile([128, S], bf16)          # v natural, t-chunk tt at [:, tt*128:]

            for _rep in range(reps):
                # ================= QKV projection =================
                with tc.tile_pool(name="wqkv", bufs=1) as wp, \
                     tc.tile_pool(name="xs", bufs=2) as xsp, \
                     tc.tile_pool(name="rtmp", bufs=3 if deep else 2) as rtp, \
                     tc.tile_pool(name="tr_ps", bufs=2, space="PSUM") as trp, \
                     tc.tile_pool(name="qkv_ps", bufs=6, space="PSUM") as qps:
                    # chunked weight + x loads, interleaved in consumption
                    # order so the first matmul starts after ~512KB
                    wq = wp.tile([128, MC * 512], bf16)
                    wk = wp.tile([128, MC * 128], bf16)
                    wv = wp.tile([128, MC * 128], bf16)
                    xs0 = xsp.tile([128, MC * 512], bf16, name="xs")
                    for mc4 in range(0, MC, 4):
                        s5 = slice(mc4 * 512, (mc4 + 4) * 512)
                        s1 = slice(mc4 * 128, (mc4 + 4) * 128)
                        dma(xs0[:, s5], xt_d[:, s5])
                        dma(wq[:, s5], wq_d[:, s5])
                        dma(wk[:, s1], wk_d[:, s1])
                        dma(wv[:, s1], wv_d[:, s1])
                    cosq = wp.tile([128, S], bf16); nc.sync.dma_start(cosq[:], cosq_d[:])
                    sinq = wp.tile([128, S], bf16); nc.sync.dma_start(sinq[:], sinq_d[:])
                    cosk = wp.tile([128, S], bf16); nc.sync.dma_start(cosk[:], cosk_d[:])
                    sink = wp.tile([128, S], bf16); nc.sync.dma_start(sink[:], sink_d[:])
                    vTt = wp.tile([128, S], bf16)   # v^T (pre-transpose)

                    def rope(dst, ps, cos_t, sin_t, sc):
                        # dst = ps*cos + swap64(ps)*sin_signed  (all [128,512])
                        # partition-crossing reads must come from PSUM: walrus
                        # rejects SB+SB tensor_tensor with mismatched base partition
                        cs = slice(sc * 512, (sc + 1) * 512)
                        t1 = rtp.tile([128, 512], f32, name="rope_t1")
                        nc.vector.tensor_tensor(t1[:], ps[:], cos_t[:, cs], mult)
                        t2 = rtp.tile([128, 512], f32, name="rope_t2")
                        nc.vector.tensor_tensor(t2[0:64, :], ps[64:128, :],
                                                sin_t[0:64, cs], mult)
                        nc.vector.tensor_tensor(t2[64:128, :], ps[0:64, :],
                                                sin_t[64:128, cs], mult)
                        nc.vector.tensor_add(dst, t1[:], t2[:])

                    def blk_lhsT(blk, mc):
                        if blk < G:
                            return wq[:, mc * 512 + 128 * blk:
                                      mc * 512 + 128 * blk + 128]
                        elif blk == G:
                            return wk[:, mc * 128:(mc + 1) * 128]
                        return wv[:, mc * 128:(mc + 1) * 128]

                    for sc in range(SC):
                        if sc == 0:
                            xs = xs0
                        else:
                            xs = xsp.tile([128, MC * 512], bf16, name="xs")
                            for mc4 in range(0, MC, 4):
                                s5 = slice(mc4 * 512, (mc4 + 4) * 512)
                                dma(
                                    xs[:, s5],
                                    xt_d[:, sc * MC * 512 + mc4 * 512:
                                         sc * MC * 512 + (mc4 + 4) * 512])
                        cs = slice(sc * 512, (sc + 1) * 512)
                        # 6 interleaved PSUM chains: PE consumes each
                        # weight/x chunk right after its DMA lands
                        pss = {}
                        for blk in range(G + 2):
                            pss[blk] = qps.tile([128, 512], f32, name="qkv_ps")
                            # bias via rank-1 matmul (contraction dim 1)
                            nc.tensor.matmul(
                                pss[blk][:],
                                bqw[:, blk * 128:(blk + 1) * 128],
                                onesr[:],
                                start=True, stop=False)
                        for mc in range(MC):
                            for blk in range(G + 2):
                                nc.tensor.matmul(
                                    pss[blk][:], blk_lhsT(blk, mc),
                                    xs[:, mc * 512:(mc + 1) * 512],
                                    start=False, stop=(mc == MC - 1))
                        # k first: attention's first QK matmul waits on kT
                        for blk in [G, G + 1] + list(range(G)):
                            ps = pss[blk]
                            if blk < G:
                                rope(qT[:, blk * S + sc * 512:
                                         blk * S + sc * 512 + 512],
                                     ps, cosq, sinq, sc)
                            elif blk == G:
                                rope(kT[:, cs], ps, cosk, sink, sc)
                            else:
                                nc.scalar.activation(vTt[:, cs], ps[:], Ident)
                        # transpose this chunk's v^T -> v natural right away so
                        # attention chunk c can start as soon as chunk c of
                        # q/k/v is done
                        for tt in range(4 * sc, 4 * sc + 4):
                            pst = trp.tile([128, 128], bf16, name="tr_ps",
                                           bufs=2)
                            nc.tensor.transpose(
                                pst[:], vTt[:, tt * 128:(tt + 1) * 128],
                                ident[:])
                            nc.vector.tensor_copy(vN[:, tt * 128:(tt + 1) * 128],
                                                  pst[:])

                # ========== attention + interleaved out-proj partials ==========
                with tc.tile_pool(name="wo_sb", bufs=1) as wop, \
                     tc.tile_pool(name="attn_sb", bufs=1) as atp:
                    wo = wop.tile([128, G * 8 * 512], bf16)
                    dma(wo[:], wo_d[:])
                    attnT = atp.tile([128, G * S], bf16)  # attn out^T
                    # bufs=2: next rep's out-proj writes don't wait for this
                    # rep's ReduceScatter to drain
                    if coll == "rs4":
                        # 4 contiguous col-chunk buffers (collectives reject
                        # strided access patterns)
                        rs_in4 = [dramp.tile([S, 1024], bf16, name=f"rs_in{p}",
                                             bufs=2) for p in range(4)]
                    else:
                        rs_in = dramp.tile([S, D], bf16, name="rs_in", bufs=2)
                    with tc.tile_pool(name="es_sb",
                                      bufs=6 if deep else 4) as esp, \
                         tc.tile_pool(name="acc_sb", bufs=2) as accp, \
                         tc.tile_pool(name="ops_sb",
                                      bufs=4 if deep else 3) as osb, \
                         tc.tile_pool(name="qk_ps", bufs=2, space="PSUM") as qkp, \
                         tc.tile_pool(name="av_ps", bufs=3, space="PSUM") as avp:
                        def outproj_piece(st, ep):
                            # a pair of e-columns per attnT weight load: the
                            # two matmuls per h share lhsT (weight reload is
                            # ~175ns on HW), accumulating into two psums
                            e0, e1 = 2 * ep, 2 * ep + 1
                            poA = avp.tile([128, 512], f32, name="av")
                            poB = avp.tile([128, 512], f32, name="av")
                            for h in range(G):
                                lw = attnT[:, h * S + st * 128:
                                           h * S + st * 128 + 128]
                                nc.tensor.matmul(
                                    poA[:], lw,
                                    wo[:, (h * 8 + e0) * 512:
                                       (h * 8 + e0) * 512 + 512],
                                    start=(h == 0), stop=(h == G - 1))
                                nc.tensor.matmul(
                                    poB[:], lw,
                                    wo[:, (h * 8 + e1) * 512:
                                       (h * 8 + e1) * 512 + 512],
                                    start=(h == 0), stop=(h == G - 1))
                            for e, po in ((e0, poA), (e1, poB)):
                                ob = osb.tile([128, 512], bf16, name="ob")
                                # GPSIMD can't read PSUM; split ACT/DVE so
                                # each runs under the PE matmul rate
                                if e % 2 == 0:
                                    nc.scalar.activation(ob[:], po[:], CopyF)
                                else:
                                    nc.vector.tensor_copy(ob[:], po[:])
                                if coll == "noout":
                                    continue  # timing probe: skip rs writes
                                if coll == "rs4":
                                    dst = rs_in4[e // 2][
                                        st * 128:(st + 1) * 128,
                                        (e % 2) * 512:(e % 2) * 512 + 512]
                                else:
                                    dst = rs_in[st * 128:(st + 1) * 128,
                                                e * 512:(e + 1) * 512]
                                nc.sync.dma_start(dst, ob[:])

                        # out-proj pieces of chunk c-1 are interleaved into
                        # chunk c's h-loop: they fill PE stalls while ACT/DVE
                        # work on the attention chain
                        for c in range(SC):
                            for h in range(G):
                                hs = h * S + c * 512
                                av = avp.tile([128, 512], f32, name="av")
                                acc = accp.tile([128, 512], bf16, name="acc")
                                ntt = 4 * (c + 1)
                                for tb in range(ntt // 2):
                                    # two t-tiles share one PSUM + one exp
                                    qk = qkp.tile([128, 1024], f32, name="qk")
                                    for q2 in range(2):
                                        tt = 2 * tb + q2
                                        # full width even on diagonal tiles:
                                        # exp must never read stale PSUM
                                        # (overflow -> inf*0 = NaN)
                                        nc.tensor.matmul(
                                            qk[:, q2 * 512:(q2 + 1) * 512],
                                            kT[:, tt * 128:(tt + 1) * 128],
                                            qT[:, hs:hs + 512],
                                            start=True, stop=True)
                                    es = esp.tile([128, 1024], bf16, name="es")
                                    nc.scalar.activation(es[:], qk[:], Exp)
                                    for q2 in range(2):
                                        tt = 2 * tb + q2
                                        if tt >= 4 * c:  # diagonal: 0/1 mask mult
                                            r = tt - 4 * c
                                            nc.vector.tensor_tensor(
                                                es[:, q2 * 512:(q2 + 1) * 512],
                                                es[:, q2 * 512:(q2 + 1) * 512],
                                                mask[:, r * 512:(r + 1) * 512],
                                                mult)
                                    for q2 in range(2):
                                        tt = 2 * tb + q2
                                        esl = es[:, q2 * 512:(q2 + 1) * 512]
                                        if tt == 0:
                                            nc.vector.tensor_copy(acc[:], esl)
                                        else:
                                            nc.vector.tensor_add(acc[:], acc[:],
                                                                 esl)
                                        nc.tensor.matmul(
                                            av[:], vN[:, tt * 128:(tt + 1) * 128],
                                            esl,
                                            start=(tt == 0), stop=(tt == ntt - 1))
                                # denom: all-ones [128,128] matmul = partition
                                # reduce + broadcast in one standard-shape op
                                bps = qkp.tile([128, 512], f32,
                                               name="bps", bufs=1)
                                nc.tensor.matmul(bps[:], onem[:], acc[:],
                                                 start=True, stop=True)
                                recb = esp.tile([128, 512], f32, name="recb",
                                                bufs=2)
                                nc.vector.reciprocal(recb[:], bps[:])
                                nc.vector.tensor_tensor(
                                    attnT[:, hs:hs + 512], av[:], recb[:], mult)
                                if c > 0:
                                    st = 4 * (c - 1) + h
                                    for ep in range(4):
                                        outproj_piece(st, ep)
                            if c == SC - 1:
                                for st in range(4 * c, 4 * c + 4):
                                    for ep in range(4):
                                        outproj_piece(st, ep)

                    # ====== ReduceScatter: route + reduce partial products ======
                    if coll == "noout":
                        nc.sync.dma_start(out_d[:], rs_in[0:SSLICE, :])
                    elif coll == "copy":
                        # single-core timing stand-in
                        nc.sync.dma_start(out_d[:], rs_in[0:SSLICE, :])
                    elif coll == "rs4":
                        for p4 in range(4):
                            ro = dramp.tile([SSLICE, 1024], bf16,
                                            name=f"rs_out{p4}", bufs=2)
                            nc.gpsimd.collective_compute(
                                "ReduceScatter", mybir.AluOpType.add,
                                replica_groups=[list(range(N_CORES))],
                                ins=[rs_in4[p4].opt()], outs=[ro.opt()])
                            nc.sync.dma_start(
                                out_d[:, p4 * 1024:(p4 + 1) * 1024], ro[:])
                    else:
                        rs_out = dramp.tile([SSLICE, D], bf16, name="rs_out",
                                            bufs=2)
                        nc.gpsimd.collective_compute(
                            "ReduceScatter", mybir.AluOpType.add,
                            replica_groups=[list(range(N_CORES))],
                            ins=[rs_in.opt()], outs=[rs_out.opt()])
                        nc.sync.dma_start(out_d[:], rs_out[:])
    nc.compile()
    return nc


def _prep_inputs(x, Wqkv, bqkv, Wo, bo):
    """Host-side shard prep. Returns in_maps for the 8 cores."""
    x0T = np.ascontiguousarray(np.asarray(x, F32)[0].T)          # [D, S]
    # [p, sc, mc, c] tiling of x^T
    xt_t = np.ascontiguousarray(
        x0T.reshape(MC, 128, SC, 512).transpose(1, 2, 0, 3)
    ).reshape(128, SC * MC * 512).astype(BF16)

    Wqkv = np.asarray(Wqkv, F32)
    Wo = np.asarray(Wo, F32)
    bqkv = np.asarray(bqkv, F32)


    # rope tables (transposed: [HD, S]); q tables fold in 1/sqrt(HD)
    inv_freq = 1.0 / (BASE ** (np.arange(0, HD, 2, dtype=np.float64) / HD))
    t = np.arange(S, dtype=np.float64)
    freqs = np.outer(t, inv_freq)                                # [S, 64]
    emb = np.concatenate([freqs, freqs], axis=1)                 # [S, HD]
    cosT = np.cos(emb).T.astype(F32)                             # [HD, S]
    sinT = np.sin(emb).T.astype(F32)
    sin_signed = np.concatenate([-sinT[:64], sinT[64:]], axis=0)
    cosq = np.ascontiguousarray(cosT * SCALE).astype(BF16)
    sinq = np.ascontiguousarray(sin_signed * SCALE).astype(BF16)
    cosk = np.ascontiguousarray(cosT).astype(BF16)
    sink = np.ascontiguousarray(sin_signed).astype(BF16)

    # multiplicative 0/1 causal masks for the 4 diagonal t-tiles of each
    # 512-wide s-chunk (applied to exp(scores) in bf16)
    p = np.arange(128)[:, None]
    f = np.arange(512)[None, :]
    mask = np.stack([(128 * r + p <= f).astype(F32) for r in range(4)],
                    axis=1).reshape(128, 4 * 512).astype(BF16)

    ident = np.eye(128, dtype=np.float32).astype(BF16)
    onem = np.ones((128, 128), BF16)

    in_maps = []
    for g in range(N_CORES):
        wq_g = np.ascontiguousarray(
            Wqkv[:, 512 * g:512 * (g + 1)].reshape(MC, 128, 512)
            .transpose(1, 0, 2)).reshape(128, MC * 512).astype(BF16)
        # own 512 Wo rows, laid out [dim-part 128, (h, e, 512 cols)]
        wo_g = np.ascontiguousarray(
            Wo[512 * g:512 * (g + 1), :].reshape(G, 128, 8, 512)
            .transpose(1, 0, 2, 3)).reshape(128, G * 8 * 512).astype(BF16)
        wk_g = np.ascontiguousarray(
            Wqkv[:, D + 128 * g:D + 128 * (g + 1)].reshape(MC, 128, 128)
            .transpose(1, 0, 2)).reshape(128, MC * 128).astype(BF16)
        wv_g = np.ascontiguousarray(
            Wqkv[:, D + KV + 128 * g:D + KV + 128 * (g + 1)]
            .reshape(MC, 128, 128).transpose(1, 0, 2)
        ).reshape(128, MC * 128).astype(BF16)
        bqw_g = np.concatenate([
            bqkv[512 * g:512 * (g + 1)],
            bqkv[D + 128 * g:D + 128 * (g + 1)],
            bqkv[D + KV + 128 * g:D + KV + 128 * (g + 1)],
        ]).reshape(1, (G + 2) * 128).astype(BF16)
        in_maps.append({
            "xt": xt_t, "wq": wq_g, "wk": wk_g, "wv": wv_g, "wo": wo_g,
            "cosq": cosq, "sinq": sinq, "cosk": cosk, "sink": sink,
            "bqw": bqw_g, "ones": np.ones((1, 512), BF16), "mask": mask,
            "ident": ident, "onem": onem,
        })
    return in_maps


def kernel(x, Wqkv, bqkv, Wo, bo):
    if "nc" not in _CACHE:
        _CACHE["nc"] = _build(reps=1)
    nc = _CACHE["nc"]
    in_maps = _prep_inputs(x, Wqkv, bqkv, Wo, bo)
    res = run_bass_kernel_spmd(nc, in_maps, core_ids=list(range(N_CORES)))
    out = np.concatenate(
        [np.asarray(res.results[g]["out"], F32) for g in range(N_CORES)], axis=0)
    out = out + np.asarray(bo, F32)[None, :]
    return out[None].astype(F32)

